# revision 1
# baseline (speedup 1.0000x reference)
"""Trainium2 8-core Bass kernel: out = sigmoid(encoder_outputs @ hidden),
encoder_outputs [32768, 1024] f32, hidden [1024] f32 -> [1, 1, 32768] f32.

Sharding: encoder_outputs splits along seq_len into 8 slices of [4096, 1024]
(one per NeuronCore); hidden is replicated; each core produces its 4096
sigmoid scores and the host concatenates. No collectives needed.

Per-core kernel (raw bacc, hand-placed semaphores; no Tile machinery):
  - partition p owns rows [32p, 32p+32) of the slice, so the [128, 32]
    result tile maps to the output vector with contiguous stores
  - hybrid stream start: SP/HWDGE issues hidden + the first 2 rows as f32
    loads before the SWDGE ring init finishes, engaging HBM early; those
    head rows are multiplied in f32
  - the remaining 30 rows stream as SWDGE cast-DMAs (f32 DRAM -> bf16
    SBUF) with tapered load sizes so compute tracks the stream closely
  - per load, one bf16 VectorEngine tensor_tensor multiplies all its rows
    against hidden (broadcast AP, DVE 2x packed mode); row reductions are
    split between a batched DVE tensor_reduce and per-row ScalarE
    activation(Copy)+accumulate, all into f32 scores
  - a warm Sigmoid on the const-zero AP makes the single ACT funcset load
    (sigmoid_and_friends covers Copy too) happen at start, off the tail
  - branch hints arm the prefetcher for each engine's end-block branch,
    avoiding the IRAM fetch stall at kernel end; the end barrier is the
    cheap sem-only variant and the final store is gated by an explicit
    semaphore wait on SP
Memory-bound at the ~360 GB/s/core HBM roofline (~47 us stream + ~7 us
fixed NEFF preamble); bf16 multiply keeps rel err ~5.6e-3 (gate 2e-2).
"""
import numpy as np
from concourse.bass_utils import run_bass_kernel_spmd


import concourse.bass as bass
from concourse import bacc, mybir


class _HintedBlock(bass.BassBlock):
    """no_gpsimd_drain block whose end-bb branches carry prefetch hints."""

    def __init__(self, bass_, name):
        super().__init__(bass_, name, no_gpsimd_drain=True)
        self.hint_locs = {}

    def __exit__(self, exc_type, exc_val, exc_tb):
        if exc_type is not None:
            return
        for engine, last_body in self.last_body.items():
            with self.bass.body(last_body, parent=self.bass.cur_bb,
                                allow_existing_parent=True):
                br = engine.br(self.end_bb)
                loc = self.hint_locs.get(engine)
                if loc is not None:
                    br.branch_hint(loc)
        self.bass.switch_bb(self.end_bb)
        gpsimd_type = self.bass.gpsimd.engine
        for eng_type, eng in self.bass.engines.items():
            if eng_type == gpsimd_type:
                continue
            d = mybir.InstDrain(
                name=self.bass.get_next_instruction_name(),
                ins=[], outs=[], bass_is_fusable=False)
            d.engine = eng_type
            eng.add_instruction(d)
        self.bass.all_engine_barrier(sem_only=True)

N_CORES = 8
SEQ = 32768
D = 1024
ROWS = SEQ // N_CORES          # 4096
RPP = ROWS // 128              # 32
F32 = mybir.dt.float32
BF16 = mybir.dt.bfloat16

HEAD_ROWS = 2                  # rows loaded f32 via HWDGE at kernel start
# SWDGE loads cover rows HEAD_ROWS..31
LOAD_SIZES = [2, 4, 4, 4, 4, 4, 4, 2, 1, 1]
DVE_ROWS = [1, 1, 1, 1, 1, 1, 1, 1, 1, 0]
OUT_SPLIT = 30


def build(load_sizes=LOAD_SIZES, dve_rows=DVE_ROWS, out_split=OUT_SPLIT,
          head_rows=HEAD_ROWS):
    assert head_rows + sum(load_sizes) == RPP
    n_loads = len(load_sizes)
    cum_rows = np.cumsum([head_rows] + list(load_sizes))  # row0 of each load
    cum_tr = np.cumsum([0] + [1 if r > 0 else 0 for r in dve_rows])
    split_load = int(np.searchsorted(cum_rows, out_split))

    nc = bacc.Bacc("TRN2", target_bir_lowering=False, debug=False,
                   num_devices=N_CORES)
    h_dram = nc.dram_tensor("hidden", [D], F32, kind="ExternalInput")
    e_dram = nc.dram_tensor("encoder_outputs", [ROWS, D], F32,
                            kind="ExternalInput")
    o_dram = nc.dram_tensor("out", [ROWS], F32, kind="ExternalOutput")
    e_view = e_dram.ap().rearrange("(p r) d -> p (r d)", p=128)
    o_view = o_dram.ap().rearrange("(p r) -> p r", p=128)

    eallf = nc.alloc_sbuf_tensor("eallf", [128, head_rows * D], F32)
    eall = nc.alloc_sbuf_tensor("eall", [128, (RPP - head_rows) * D], BF16)
    htf = nc.alloc_sbuf_tensor("htf", [128, D], F32)
    ht = nc.alloc_sbuf_tensor("ht", [128, D], BF16)
    prodf = nc.alloc_sbuf_tensor("prodf", [128, head_rows * D], BF16)
    prods = [nc.alloc_sbuf_tensor(f"prod{i}", [128, sz * D], BF16)
             for i, sz in enumerate(load_sizes)]
    scores = nc.alloc_sbuf_tensor("scores", [128, RPP], F32)
    sig = nc.alloc_sbuf_tensor("sigout", [128, RPP], F32)

    head_sem = nc.alloc_semaphore("hd")
    load_sems = [nc.alloc_semaphore(f"ld{i}") for i in range(n_loads)]
    h_sem = nc.alloc_semaphore("hld")
    tt_sem = nc.alloc_semaphore("tt")
    tr_sem = nc.alloc_semaphore("tr")
    sig_sem = nc.alloc_semaphore("sg")
    outd_sem = nc.alloc_semaphore("outd")

    with _HintedBlock(nc, f"blk{nc.next_id()}") as block:

        @block.gpsimd
        def _(g: bass.BassEngine):
            block.hint_locs[g] = g.mark_branch_hint_location()
            for i, sz in enumerate(load_sizes):
                r0 = int(cum_rows[i])
                g.dma_start(
                    out=eall.ap()[:, (r0 - head_rows) * D:
                                  (r0 - head_rows + sz) * D],
                    in_=e_view[:, r0 * D:(r0 + sz) * D],
                ).then_inc(load_sems[i], 16)

        @block.vector
        def _(v: bass.BassEngine):
            block.hint_locs[v] = v.mark_branch_hint_location()
            # head rows in f32 (htf direct, no cast dependency)
            v.wait_ge(h_sem, 16)
            v.wait_ge(head_sem, 16)
            v.tensor_tensor(
                out=prodf.ap().rearrange("p (r d) -> p r d", r=head_rows),
                in0=eallf.ap().rearrange("p (r d) -> p r d", r=head_rows),
                in1=htf.ap().unsqueeze(1).broadcast_to((128, head_rows, D)),
                op=mybir.AluOpType.mult,
            ).then_inc(tt_sem, 1)
            v.tensor_copy(out=ht.ap(), in_=htf.ap())
            for i, sz in enumerate(load_sizes):
                r0 = int(cum_rows[i])
                r = dve_rows[i]
                slot = prods[i]
                v.wait_ge(load_sems[i], 16)
                v.tensor_tensor(
                    out=slot.ap().rearrange("p (r d) -> p r d", r=sz),
                    in0=eall.ap()[:, (r0 - head_rows) * D:
                                  (r0 - head_rows + sz) * D]
                        .rearrange("p (r d) -> p r d", r=sz),
                    in1=ht.ap().unsqueeze(1).broadcast_to((128, sz, D)),
                    op=mybir.AluOpType.mult,
                ).then_inc(tt_sem, 1)
                if r > 0:
                    v.tensor_reduce(
                        out=scores.ap()[:, r0:r0 + r],
                        in_=slot.ap()[:, :r * D].rearrange(
                            "p (r d) -> p r d", r=r),
                        axis=mybir.AxisListType.X, op=mybir.AluOpType.add,
                    ).then_inc(tr_sem, 1)

        @block.scalar
        def _(s: bass.BassEngine):
            block.hint_locs[s] = s.mark_branch_hint_location()
            # warm the sigmoid_and_friends funcset (covers Copy too) so the
            # tail sigmoid doesn't trigger a second ACT table load
            cz = nc.const_aps.scalar_like(0.0, sig.ap()[:, 0:1])
            s.activation(out=sig.ap()[:, 0:1], in_=cz,
                         func=mybir.ActivationFunctionType.Sigmoid)
            # head-row reduces (read bf16 products of the f32 head TT)
            s.wait_ge(tt_sem, 1)
            for j in range(head_rows):
                s.activation(
                    out=prodf.ap()[:, j * D:(j + 1) * D],
                    in_=prodf.ap()[:, j * D:(j + 1) * D],
                    func=mybir.ActivationFunctionType.Copy,
                    accum_out=scores.ap()[:, j:j + 1],
                )
            for i, sz in enumerate(load_sizes):
                r0 = int(cum_rows[i])
                r = dve_rows[i]
                slot = prods[i]
                if r < sz:
                    s.wait_ge(tt_sem, i + 2)
                for j in range(r, sz):
                    col = r0 + j
                    s.activation(
                        out=slot.ap()[:, j * D:(j + 1) * D],
                        in_=slot.ap()[:, j * D:(j + 1) * D],
                        func=mybir.ActivationFunctionType.Copy,
                        accum_out=scores.ap()[:, col:col + 1],
                    )
                if i + 1 == split_load:
                    s.wait_ge(tr_sem, int(cum_tr[i + 1]))
                    s.activation(
                        out=sig.ap()[:, :out_split],
                        in_=scores.ap()[:, :out_split],
                        func=mybir.ActivationFunctionType.Sigmoid,
                    ).then_inc(sig_sem, 1)
            s.wait_ge(tr_sem, int(cum_tr[n_loads]))
            s.activation(
                out=sig.ap()[:, out_split:], in_=scores.ap()[:, out_split:],
                func=mybir.ActivationFunctionType.Sigmoid,
            ).then_inc(sig_sem, 1)

        @block.sync
        def _(sy: bass.BassEngine):
            block.hint_locs[sy] = sy.mark_branch_hint_location()
            # hidden first (TTs need it), then the f32 head rows — all HWDGE,
            # issued before the SWDGE ring finishes initializing
            sy.dma_start(
                out=htf.ap(),
                in_=h_dram.ap().unsqueeze(0).broadcast_to((128, D))
            ).then_inc(h_sem, 16)
            sy.dma_start(out=eallf.ap(),
                         in_=e_view[:, 0:head_rows * D]).then_inc(head_sem, 16)
            sy.wait_ge(sig_sem, 1)
            sy.dma_start(out=o_view[:, :out_split],
                         in_=sig.ap()[:, :out_split]).then_inc(outd_sem, 16)
            sy.wait_ge(sig_sem, 2)
            sy.dma_start(out=o_view[:, out_split:],
                         in_=sig.ap()[:, out_split:]).then_inc(outd_sem, 16)
            sy.wait_ge(outd_sem, 32)

    nc.compile()
    return nc


def make_in_maps(hidden, encoder_outputs):
    hidden = np.ascontiguousarray(np.asarray(hidden, dtype=np.float32))
    encoder_outputs = np.asarray(encoder_outputs, dtype=np.float32)
    return [
        {"hidden": hidden,
         "encoder_outputs": np.ascontiguousarray(
             encoder_outputs[i * ROWS:(i + 1) * ROWS])}
        for i in range(N_CORES)
    ]


_NC_CACHE = None


def _get_nc():
    global _NC_CACHE
    if _NC_CACHE is None:
        _NC_CACHE = build()
    return _NC_CACHE


def _make_in_maps(hidden, encoder_outputs):
    return make_in_maps(hidden, encoder_outputs)


def kernel(hidden, encoder_outputs):
    nc = _get_nc()
    in_maps = make_in_maps(hidden, encoder_outputs)
    res = run_bass_kernel_spmd(nc, in_maps, core_ids=list(range(N_CORES)))
    out = np.concatenate(
        [np.asarray(res.results[i]["out"]).reshape(-1) for i in range(N_CORES)])
    return out[None, None, :].astype(np.float32)

